# revision 13
# baseline (speedup 1.0000x reference)
"""Trainium2 Bass kernel for nn_AttentionSubsample (8-core SPMD).

Sharding: batch N=2 x 4 head-groups (3 heads each) -> 8 cores, no
collectives.  Each core computes q/k/v projections for its head group
(K/V on the stride-2 subsampled positions only), per-head attention with
softmax folded as exp -> denominator via an appended ones-column in V ->
divide, and its partial output projection in transposed layout.  The
host sums the 4 per-batch partials and adds the bias.

Schedule notes (the kernel is jointly ACT(exp)- and PE-bound):
 - q chunks are 512 wide (one PSUM bank per matmul): at F=512 the
   213 ns score matmul hides the 107 ns LDWEIGHTS of the opposite
   row-half, so the (even,odd) k-tile pairs overlap fully on the PE.
   The ragged 64-wide tail chunk keeps the pipeline drain short.
 - x^T DMAs first, issued from two engine queues (the sync sequencer
   alone takes ~0.7us per dma_start); weights follow.
 - All projections except the first kT chunk run as fillers between
   score groups; a junk-matmul warmup keeps the PE clock at 8/8 during
   the initial x DMA.
 - P@V for the previous chunk pair runs as 26-matmul contiguous
   bursts (amortizes the PE stream-switch penalty).
 - Softmax normalization: DVE reciprocal of the denominator row (via
   an SBUF copy - custom-DVE bit tricks must not read PSUM), GpSimd
   partition-broadcast, one DVE multiply -> normalized bf16 attention.

Layout notes:
 - The spatial stride-2 subsample of K/V equals taking even rows of the
   flattened [3136, 768] batch (196 is even), i.e. even columns of x^T.
 - All device matmuls run in bf16 (fp32 PSUM accumulation).
 - Output is written bf16 (partial sums; host accumulates in fp32).
"""

import sys

for _p in ("/opt/trn_rl_repo",):
    if _p not in sys.path:
        sys.path.insert(0, _p)

import numpy as np
import ml_dtypes

import concourse.bass as bass  # noqa: F401  (registers engines)
import concourse.tile as tile
from concourse import bacc, mybir
from concourse.bass_utils import run_bass_kernel_spmd

BFNP = ml_dtypes.bfloat16
F32 = mybir.dt.float32
BF16 = mybir.dt.bfloat16
AF = mybir.ActivationFunctionType

N, T, S, D = 2, 16, 196, 768
H, HD = 12, 64
Q = T * S              # 3136 query positions per batch
KP = T * (S // 2)      # 1568 subsampled key positions
HPG = 3                # heads per group (12 heads / 4 groups)
GD = HPG * HD          # 192 channels per head group
SC = (D // H) ** -0.5  # 0.125 attention scale
NKT = 13               # k tiles: 12 * 128 + 32
KTL = 32               # last k-tile height
NDK = D // 128         # 6 contraction tiles for the projections
N_CORES = 8
N_WARM = 36            # junk matmuls covering the x DMA (HAM warmup)
ESW = 512              # es slot width (q per chunk)

# q chunks: one 64-wide head (absorbs raggedness during the x DMA)
# then six 512-wide
CHS = [(0, 64)] + [(64 + c * 512, 512) for c in range(6)]
NCH = len(CHS)
CPAIRS = [(0,), (1, 2), (3, 4), (5, 6)]
# exp groups over k-tiles: PSUM scores tile holds 2 banks (512-aligned)
EXP_GROUPS = [(0, 1), (2, 3), (4, 5), (6, 7), (8, 9), (10, 11), (12,)]
# key chunks for the kT m0 projection
KCH = [(0, 512), (512, 512), (1024, 512), (1536, 32)]

TRACE = False          # test.py flips this for profiled runs
LAST_RESULTS = {}      # exec_time_ns etc. stashed here on traced runs

_CACHE = {}


def _ksize(kt):
    return 128 if kt < NKT - 1 else KTL


def _head_pos(h):
    """(block, partition base) of head h inside the 2-block qT/kT tiles."""
    return (0, 0) if h == 0 else ((0, 64) if h == 1 else (1, 0))


def _build_nc():
    nc = bacc.Bacc(
        "TRN2", target_bir_lowering=False, debug=False, num_devices=N_CORES
    )
    xT = nc.dram_tensor("xT", [D, Q], BF16, kind="ExternalInput").ap()
    wq = nc.dram_tensor("wq", [D, GD], BF16, kind="ExternalInput").ap()
    wk = nc.dram_tensor("wk", [D, GD], BF16, kind="ExternalInput").ap()
    wv = nc.dram_tensor("wv", [D, GD], BF16, kind="ExternalInput").ap()
    wp = nc.dram_tensor("wp", [GD, D], BF16, kind="ExternalInput").ap()
    out = nc.dram_tensor("out", [D, Q], BF16, kind="ExternalOutput").ap()

    with tile.TileContext(nc) as tc:
        _body(tc, xT, wq, wk, wv, wp, out)
    nc.compile()
    return nc


def _body(tc, xT, wq, wk, wv, wp, out):
    nc = tc.nc
    with (
        tc.tile_pool(name="persist", bufs=1) as P,
        tc.tile_pool(name="es", bufs=3) as ES,
        tc.tile_pool(name="inv", bufs=2) as INV,
        tc.tile_pool(name="ot", bufs=3) as OT,
        tc.tile_pool(name="scps", bufs=3, space="PSUM") as SCPS,
        tc.tile_pool(name="numps", bufs=2, space="PSUM") as NUMPS,
    ):
        # ---- persistent SBUF tensors -------------------------------------
        # x^T gates everything: DMA it first, issue from two engine queues.
        junk_w = P.tile([128, 128], BF16, tag="jw")
        nc.gpsimd.memset(junk_w[:], 0.0)
        junk_m = P.tile([128, ESW], BF16, tag="jm")
        nc.gpsimd.memset(junk_m[:], 0.0)
        xt = P.tile([128, NDK * Q], BF16, tag="xt")
        _xteng = [nc.sync, nc.scalar, nc.gpsimd]
        _i = 0
        for s0, sl in ((0, 1088), (1088, 1024), (2112, 1024)):
            for kt in range(NDK):
                _xteng[_i % 3].dma_start(
                    xt[:, kt * Q + s0 : kt * Q + s0 + sl],
                    xT[kt * 128 : (kt + 1) * 128, s0 : s0 + sl],
                )
                _i += 1
        wk_sb = P.tile([128, NDK * 128], BF16, tag="wk")
        nc.sync.dma_start(
            wk_sb[:].rearrange("p (a g) -> p a g", a=NDK),
            wk.rearrange("(a p) g -> p a g", p=128)[:, :, 0:128],
        )
        # combined m1 weights: cols [wq_h2 | wk_h2] per D-ktile
        wqk2 = P.tile([128, NDK * 128], BF16, tag="wqk2")
        nc.gpsimd.dma_start(
            wqk2[:].rearrange("p (a g) -> p a g", a=NDK)[:, :, 0:HD],
            wq.rearrange("(a p) g -> p a g", p=128)[:, :, 2 * HD : GD],
        )
        nc.gpsimd.dma_start(
            wqk2[:].rearrange("p (a g) -> p a g", a=NDK)[:, :, HD:128],
            wk.rearrange("(a p) g -> p a g", p=128)[:, :, 2 * HD : GD],
        )
        wq_sb = P.tile([128, NDK * 128], BF16, tag="wq")
        nc.sync.dma_start(
            wq_sb[:].rearrange("p (a g) -> p a g", a=NDK),
            wq.rearrange("(a p) g -> p a g", p=128)[:, :, 0:128],
        )
        wv_sb = P.tile([128, NDK * GD], BF16, tag="wv")
        nc.gpsimd.dma_start(
            wv_sb[:].rearrange("p (a g) -> p a g", a=NDK),
            wv.rearrange("(a p) g -> p a g", p=128),
        )
        # wp: h0 rows at partitions 0:64 and h1 at 64:128 of block 0 (so the
        # h0+h1 pair contracts as one K=128 matmul); h2 in block 1.
        wp_sb = P.tile([128, 2 * D], BF16, tag="wp")
        nc.sync.dma_start(wp_sb[0:HD, 0:D], wp[0:HD, :])
        nc.sync.dma_start(wp_sb[HD:128, 0:D], wp[HD : 2 * HD, :])
        nc.sync.dma_start(wp_sb[0:HD, D : 2 * D], wp[2 * HD : 3 * HD, :])

        qT = P.tile([128, 2 * Q], BF16, tag="qT")       # q^T: rows=[h0|h1], [h2]
        kT = P.tile([128, 2 * KP], BF16, tag="kT")      # k^T subsampled
        qT_dup = P.tile([128, 2 * Q], BF16, tag="qTd")  # row-halves swapped
        kT_dup = P.tile([128, 2 * KP], BF16, tag="kTd")
        vv = P.tile([128, HPG * NKT * 65], BF16, tag="v")  # v + ones col, [k, 65]/tile
        # attn out: block 0 rows 0:64 = h0, rows 64:128 = h1; block 1 = h2
        attn = P.tile([128, 2 * Q], BF16, tag="attn")

        # ones columns of the v tiles
        nc.vector.memset(vv[:, 64 : HPG * NKT * 65 : 65], 1.0)

        # ---- HAM warmup: junk matmuls with no DMA dependency -------------
        jps = NUMPS.tile([128, 512], F32, tag="num", name="jps")
        for _ in range(N_WARM):
            nc.tensor.matmul(
                jps[0:128, 0:512], junk_w[:], junk_m[:], start=True, stop=True
            )

        # ---- A: projections --------------------------------------------
        def a1_chunk(i):
            """kT m0-block (heads h0|h1) for one key chunk."""
            c0, csz = KCH[i]
            ps = NUMPS.tile([128, 512], F32, tag="num", name=f"km0_{i}")
            for kt in range(NDK):
                base = kt * Q
                nc.tensor.matmul(
                    ps[0:128, 0:csz],
                    wk_sb[:, kt * 128 : (kt + 1) * 128],
                    xt[:, base + 2 * c0 : base + 2 * (c0 + csz) : 2],
                    start=(kt == 0),
                    stop=(kt == NDK - 1),
                )
            nc.vector.tensor_copy(kT[0:128, c0 : c0 + csz], ps[0:128, 0:csz])
            nc.gpsimd.dma_start(
                kT_dup[HD:128, c0 : c0 + csz], kT[0:HD, c0 : c0 + csz]
            )
            nc.gpsimd.dma_start(
                kT_dup[0:HD, c0 : c0 + csz], kT[HD:128, c0 : c0 + csz]
            )

        def a2_chunk(c):
            """Combined (q_h2 | k_h2) pass over one q chunk; q keeps all
            positions, k keeps the even ones."""
            qo, ql = CHS[c]
            ps = NUMPS.tile([128, 512], F32, tag="num", name=f"qk2_{c}")
            for kt in range(NDK):
                nc.tensor.matmul(
                    ps[0:128, 0:ql],
                    wqk2[:, kt * 128 : (kt + 1) * 128],
                    xt[:, kt * Q + qo : kt * Q + qo + ql],
                    start=(kt == 0),
                    stop=(kt == NDK - 1),
                )
            nc.vector.tensor_copy(
                qT[0:HD, Q + qo : Q + qo + ql], ps[0:HD, 0:ql]
            )
            ko, kl = qo // 2, ql // 2
            nc.vector.tensor_copy(
                kT[0:HD, KP + ko : KP + ko + kl], ps[HD:128, 0:ql:2]
            )
            nc.gpsimd.dma_start(
                qT_dup[HD:128, Q + qo : Q + qo + ql],
                qT[0:HD, Q + qo : Q + qo + ql],
            )
            nc.gpsimd.dma_start(
                kT_dup[HD:128, KP + ko : KP + ko + kl],
                kT[0:HD, KP + ko : KP + ko + kl],
            )

        def qT_m0(c):
            """qT m0-block (heads h0|h1) for one q-chunk, plus dup swaps."""
            qo, ql = CHS[c]
            ps = NUMPS.tile([128, 512], F32, tag="num", name=f"qm0_{c}")
            for kt in range(NDK):
                nc.tensor.matmul(
                    ps[0:128, 0:ql],
                    wq_sb[:, kt * 128 : (kt + 1) * 128],
                    xt[:, kt * Q + qo : kt * Q + qo + ql],
                    start=(kt == 0),
                    stop=(kt == NDK - 1),
                )
            nc.vector.tensor_copy(qT[0:128, qo : qo + ql], ps[0:128, 0:ql])
            nc.gpsimd.dma_start(
                qT_dup[HD:128, qo : qo + ql], qT[0:HD, qo : qo + ql]
            )
            nc.gpsimd.dma_start(
                qT_dup[0:HD, qo : qo + ql], qT[HD:128, qo : qo + ql]
            )

        def vv_group(kt_m):
            """v projection for one k-tile: [ksz, GD] = x_sub @ wv."""
            msz = _ksize(kt_m)
            ps = NUMPS.tile([128, 512], F32, tag="num", name=f"vv{kt_m}")
            for kt in range(NDK):
                base = kt * Q + 2 * (kt_m * 128)
                nc.tensor.matmul(
                    ps[0:msz, 0:GD],
                    xt[:, base : base + 2 * msz : 2],
                    wv_sb[:, kt * GD : (kt + 1) * GD],
                    start=(kt == 0),
                    stop=(kt == NDK - 1),
                )
            nc.vector.tensor_copy(
                vv[0:msz].rearrange("p (h s) -> p h s", h=HPG)[
                    :, :, kt_m * 65 : kt_m * 65 + 64
                ],
                ps[0:msz, 0:GD].rearrange("p (h s) -> p h s", h=HPG),
            )

        a1_chunk(0)
        qT_m0(0)
        qT_m0(1)

        # ---- B/C: attention over chunk pairs ----------------------------
        def es4(es_t):
            return es_t.rearrange("p (k j c) -> p k j c", k=NKT, j=2)

        def score_group(h, c, j, grp, es_t):
            """Scores for one (even,odd) k-tile group x one q-chunk, + exp."""
            qo, ql = CHS[c]
            blk, pb = _head_pos(h)
            scp = SCPS.tile([128, 1024], F32, tag="sc")
            pmax = _ksize(grp[-1])
            for i, kt in enumerate(grp):
                ksz = _ksize(kt)
                if kt % 2 == 0:
                    sk, sq, base = kT, qT, pb
                else:
                    sk, sq, base = kT_dup, qT_dup, HD - pb
                nc.tensor.matmul(
                    scp[0:ksz, i * 512 : i * 512 + ql],
                    sk[base : base + HD, blk * KP + kt * 128 : blk * KP + kt * 128 + ksz],
                    sq[base : base + HD, blk * Q + qo : blk * Q + qo + ql],
                    start=True,
                    stop=True,
                    tile_position=(base, 0),
                )
            e = es4(es_t)
            if len(grp) == 2:
                src_ = scp[0:pmax, 0:1024].rearrange("p (a b) -> p a b", b=512)[
                    :, :, 0:ql
                ]
                dst = e[0:pmax, grp[0] : grp[0] + 2, j, 0:ql]
            else:
                src_ = scp[0:pmax, 0:ql]
                dst = e[0:pmax, grp[0], j, 0:ql]
            nc.scalar.activation(dst, src_, AF.Exp, scale=SC)

        def pv_part(h, es_t, j, c):
            """P@V with ones-column denominator; reciprocal + broadcast."""
            ql = CHS[c][1]
            e = es4(es_t)
            num = NUMPS.tile([128, 512], F32, tag="num")
            for kt in range(NKT):
                ksz = _ksize(kt)
                slot = (h * NKT + kt) * 65
                nc.tensor.matmul(
                    num[0:65, 0:ql],
                    vv[0:ksz, slot : slot + 65],
                    e[0:ksz, kt, j, 0:ql],
                    start=(kt == 0),
                    stop=(kt == NKT - 1),
                )
            den = INV.tile([128, ESW], F32, tag="den")
            nc.vector.tensor_copy(den[0:1, 0:ql], num[64:65, 0:ql])
            inv = INV.tile([128, ESW], F32, tag="inv")
            nc.vector.reciprocal_approx_fast(inv[0:1, 0:ql], den[0:1, 0:ql])
            invb = INV.tile([128, ESW], F32, tag="invb", bufs=2)
            nc.gpsimd.partition_broadcast(
                invb[0:HD, 0:ql], inv[0:1, 0:ql], channels=HD
            )
            return num, invb

        def rep_mult(h, c, num, invb):
            qo, ql = CHS[c]
            if h == 0:
                dst = attn[0:64, qo : qo + ql]
            elif h == 1:
                dst = attn[64:128, qo : qo + ql]
            else:
                dst = attn[0:64, Q + qo : Q + qo + ql]
            nc.vector.tensor_tensor(
                dst, num[0:64, 0:ql], invb[0:HD, 0:ql], op=mybir.AluOpType.mult
            )

        def proj_one(c, m):
            qo, ql = CHS[c]
            pp = NUMPS.tile([128, 512], F32, tag="num", name=f"pj{m}")
            nc.tensor.matmul(
                pp[0:128, 0:ql],
                wp_sb[0:128, m * 128 : (m + 1) * 128],
                attn[0:128, qo : qo + ql],
                start=True,
                stop=False,
            )
            nc.tensor.matmul(
                pp[0:128, 0:ql],
                wp_sb[0:HD, D + m * 128 : D + (m + 1) * 128],
                attn[0:HD, Q + qo : Q + qo + ql],
                start=False,
                stop=True,
            )
            ot = OT.tile([128, ESW], BF16, tag="ot")
            nc.vector.tensor_copy(ot[0:128, 0:ql], pp[0:128, 0:ql])
            nc.sync.dma_start(
                out[m * 128 : (m + 1) * 128, qo : qo + ql], ot[0:128, 0:ql]
            )

        def mk(f, *a):
            return lambda: f(*a)

        prev = None  # (chunks, es tiles) of the previous pair
        for p, chunks in enumerate(CPAIRS):
            es = [
                ES.tile([128, NKT * 2 * ESW], BF16, tag="es", name=f"es{p}_{h}")
                for h in range(HPG)
            ]
            fillers = []
            if p == 0:
                # need-driven order: A1 chunks before the score groups that
                # read them (emission order IS the dependency order); within
                # that, earlier-arriving x-stripes first
                fillers += [mk(a2_chunk, 0), mk(a2_chunk, 1), mk(a1_chunk, 1)]
                fillers += [mk(a2_chunk, 2), mk(vv_group, 0), mk(vv_group, 1)]
                fillers += [mk(a1_chunk, 2), mk(a1_chunk, 3)]
                fillers += [mk(vv_group, 2), mk(vv_group, 3)]
                fillers += [mk(qT_m0, c) for c in CPAIRS[1]]
                fillers += [mk(a2_chunk, 3), mk(a2_chunk, 4)]
                fillers += [mk(vv_group, k) for k in range(4, 8)]
                fillers += [mk(a2_chunk, 5), mk(a2_chunk, 6)]
                fillers += [mk(vv_group, k) for k in range(8, NKT)]
            else:
                state = {}
                pchunks, pes_l = prev
                # h0's first P@V burst runs before the first score group so
                # the es ring buffer frees before this pair's exps land
                state[(0, 0)] = pv_part(0, pes_l[0], 0, pchunks[0])

                def mk_pv(hh, pes, jj, cc):
                    def f():
                        state[(hh, jj)] = pv_part(hh, pes, jj, cc)
                    return f

                def mk_rep(hh, cc, jj):
                    def f():
                        num, invb = state[(hh, jj)]
                        rep_mult(hh, cc, num, invb)
                    return f

                nxt = CPAIRS[p + 1] if p + 1 < len(CPAIRS) else ()
                for j, c in enumerate(pchunks):
                    if j > 0:
                        fillers.append(mk_pv(0, pes_l[0], j, c))
                for j, c in enumerate(pchunks):
                    fillers.append(mk_rep(0, c, j))
                for h in (1, 2):
                    if h - 1 < len(nxt):
                        fillers.append(mk(qT_m0, nxt[h - 1]))
                    for j, c in enumerate(pchunks):
                        fillers.append(mk_pv(h, pes_l[h], j, c))
                    for j, c in enumerate(pchunks):
                        fillers.append(mk_rep(h, c, j))
                for j, c in enumerate(pchunks):
                    for m in range(NDK):
                        fillers.append(mk(proj_one, c, m))
            fi = 0
            take = 2 if len(chunks) == 1 else 1
            for j, c in enumerate(chunks):
                for h in range(HPG):
                    for grp in EXP_GROUPS:
                        score_group(h, c, j, grp, es[h])
                        for _ in range(take):
                            if fi < len(fillers):
                                fillers[fi]()
                                fi += 1
            while fi < len(fillers):
                fillers[fi]()
                fi += 1
            prev = (chunks, es)

        # tail: PV + normalize + project the final chunk pair
        pchunks, pes_l = prev
        for h in range(HPG):
            st = [pv_part(h, pes_l[h], j, c) for j, c in enumerate(pchunks)]
            for j, c in enumerate(pchunks):
                rep_mult(h, c, *st[j])
        for j, c in enumerate(pchunks):
            for m in range(NDK):
                proj_one(c, m)


def _get_nc():
    if "nc" not in _CACHE:
        _CACHE["nc"] = _build_nc()
    return _CACHE["nc"]


def kernel(x, W_qkv, W_proj, b_proj):
    nc = _get_nc()
    xTs = [
        np.ascontiguousarray(
            x[n].reshape(Q, D).astype(BFNP).T
        )
        for n in range(N)
    ]
    wqs, wks, wvs, wps = [], [], [], []
    for g in range(4):
        c0 = g * GD
        wqs.append(np.ascontiguousarray(W_qkv[:, c0 : c0 + GD].astype(BFNP)))
        wks.append(np.ascontiguousarray(W_qkv[:, D + c0 : D + c0 + GD].astype(BFNP)))
        wvs.append(
            np.ascontiguousarray(W_qkv[:, 2 * D + c0 : 2 * D + c0 + GD].astype(BFNP))
        )
        wps.append(np.ascontiguousarray(W_proj[c0 : c0 + GD, :].astype(BFNP)))
    in_maps = [
        {"xT": xTs[c // 4], "wq": wqs[c % 4], "wk": wks[c % 4],
         "wv": wvs[c % 4], "wp": wps[c % 4]}
        for c in range(N_CORES)
    ]
    res = run_bass_kernel_spmd(nc, in_maps, list(range(N_CORES)), trace=TRACE)
    if TRACE:
        LAST_RESULTS["exec_time_ns"] = res.exec_time_ns
        LAST_RESULTS["mean_exec_time_ns"] = res.mean_exec_time_ns
    out = np.empty((N, T, S, D), np.float32)
    for n in range(N):
        acc = res.results[4 * n]["out"].astype(np.float32)
        for g in range(1, 4):
            acc = acc + res.results[4 * n + g]["out"].astype(np.float32)
        out[n] = (acc.T + b_proj).reshape(T, S, D)
    return out


# revision 14
# speedup vs baseline: 1.0956x; 1.0956x over previous
"""Trainium2 Bass kernel for nn_AttentionSubsample (8-core SPMD).

Sharding: batch N=2 x 4 head-groups (3 heads each) -> 8 cores, no
collectives.  Each core computes q/k/v projections for its head group
(K/V on the stride-2 subsampled positions only), per-head attention with
softmax folded as exp -> denominator via an appended ones-column in V ->
divide, and its partial output projection in transposed layout.  The
host sums the 4 per-batch partials and adds the bias.

Schedule notes (the kernel is jointly ACT(exp)- and PE-bound):
 - q chunks are 512 wide (one PSUM bank per matmul): at F=512 the
   213 ns score matmul hides the 107 ns LDWEIGHTS of the opposite
   row-half, so the (even,odd) k-tile pairs overlap fully on the PE.
   The ragged 64-wide tail chunk keeps the pipeline drain short.
 - x^T DMAs first, issued from two engine queues (the sync sequencer
   alone takes ~0.7us per dma_start); weights follow.
 - All projections except the first kT chunk run as fillers between
   score groups; a junk-matmul warmup keeps the PE clock at 8/8 during
   the initial x DMA.
 - P@V for the previous chunk pair runs as 26-matmul contiguous
   bursts (amortizes the PE stream-switch penalty).
 - Softmax normalization: DVE reciprocal of the denominator row (via
   an SBUF copy - custom-DVE bit tricks must not read PSUM), GpSimd
   partition-broadcast, one DVE multiply -> normalized bf16 attention.

Layout notes:
 - The spatial stride-2 subsample of K/V equals taking even rows of the
   flattened [3136, 768] batch (196 is even), i.e. even columns of x^T.
 - All device matmuls run in bf16 (fp32 PSUM accumulation).
 - Output is written bf16 (partial sums; host accumulates in fp32).
"""

import sys

for _p in ("/opt/trn_rl_repo",):
    if _p not in sys.path:
        sys.path.insert(0, _p)

import numpy as np
import ml_dtypes

import concourse.bass as bass  # noqa: F401  (registers engines)
import concourse.tile as tile
from concourse import bacc, mybir
from concourse.bass_utils import run_bass_kernel_spmd

BFNP = ml_dtypes.bfloat16
F32 = mybir.dt.float32
BF16 = mybir.dt.bfloat16
AF = mybir.ActivationFunctionType

N, T, S, D = 2, 16, 196, 768
H, HD = 12, 64
Q = T * S              # 3136 query positions per batch
KP = T * (S // 2)      # 1568 subsampled key positions
HPG = 3                # heads per group (12 heads / 4 groups)
GD = HPG * HD          # 192 channels per head group
SC = (D // H) ** -0.5  # 0.125 attention scale
NKT = 13               # k tiles: 12 * 128 + 32
KTL = 32               # last k-tile height
NDK = D // 128         # 6 contraction tiles for the projections
N_CORES = 8
N_WARM = 52            # junk matmuls covering the x DMA (HAM warmup)
ESW = 448              # es slot width (q per chunk)

# q chunks
CHS = [(c * 448, 448) for c in range(7)]
NCH = len(CHS)
CPAIRS = [(0, 1), (2, 3), (4, 5), (6,)]
# exp groups over k-tiles: PSUM scores tile holds 2 banks (512-aligned)
EXP_GROUPS = [(0, 1), (2, 3), (4, 5), (6, 7), (8, 9), (10, 11), (12,)]
# key chunks for the kT m0 projection
KCH = [(0, 448), (448, 448), (896, 448), (1344, 224)]

TRACE = False          # test.py flips this for profiled runs
LAST_RESULTS = {}      # exec_time_ns etc. stashed here on traced runs

_CACHE = {}


def _ksize(kt):
    return 128 if kt < NKT - 1 else KTL


def _head_pos(h):
    """(block, partition base) of head h inside the 2-block qT/kT tiles."""
    return (0, 0) if h == 0 else ((0, 64) if h == 1 else (1, 0))


def _build_nc():
    nc = bacc.Bacc(
        "TRN2", target_bir_lowering=False, debug=False, num_devices=N_CORES
    )
    xT = nc.dram_tensor("xT", [D, Q], BF16, kind="ExternalInput").ap()
    wq = nc.dram_tensor("wq", [D, GD], BF16, kind="ExternalInput").ap()
    wk = nc.dram_tensor("wk", [D, GD], BF16, kind="ExternalInput").ap()
    wv = nc.dram_tensor("wv", [D, GD], BF16, kind="ExternalInput").ap()
    wp = nc.dram_tensor("wp", [GD, D], BF16, kind="ExternalInput").ap()
    out = nc.dram_tensor("out", [D, Q], BF16, kind="ExternalOutput").ap()

    with tile.TileContext(nc) as tc:
        _body(tc, xT, wq, wk, wv, wp, out)
    nc.compile()
    return nc


def _body(tc, xT, wq, wk, wv, wp, out):
    nc = tc.nc
    with (
        tc.tile_pool(name="persist", bufs=1) as P,
        tc.tile_pool(name="es", bufs=4) as ES,
        tc.tile_pool(name="inv", bufs=2) as INV,
        tc.tile_pool(name="ot", bufs=3) as OT,
        tc.tile_pool(name="scps", bufs=3, space="PSUM") as SCPS,
        tc.tile_pool(name="numps", bufs=2, space="PSUM") as NUMPS,
    ):
        # ---- persistent SBUF tensors -------------------------------------
        # x^T gates everything: DMA it first, issue from two engine queues.
        xt = P.tile([128, NDK * Q], BF16, tag="xt")
        for kt in range(NDK):
            for hh in range(2):
                nc.sync.dma_start(
                    xt[:, kt * Q + hh * (Q // 2) : kt * Q + (hh + 1) * (Q // 2)],
                    xT[kt * 128 : (kt + 1) * 128, hh * (Q // 2) : (hh + 1) * (Q // 2)],
                )
        wk_sb = P.tile([128, NDK * 128], BF16, tag="wk")
        nc.sync.dma_start(
            wk_sb[:].rearrange("p (a g) -> p a g", a=NDK),
            wk.rearrange("(a p) g -> p a g", p=128)[:, :, 0:128],
        )
        # combined m1 weights: cols [wq_h2 | wk_h2] per D-ktile
        wqk2 = P.tile([128, NDK * 128], BF16, tag="wqk2")
        nc.gpsimd.dma_start(
            wqk2[:].rearrange("p (a g) -> p a g", a=NDK)[:, :, 0:HD],
            wq.rearrange("(a p) g -> p a g", p=128)[:, :, 2 * HD : GD],
        )
        nc.gpsimd.dma_start(
            wqk2[:].rearrange("p (a g) -> p a g", a=NDK)[:, :, HD:128],
            wk.rearrange("(a p) g -> p a g", p=128)[:, :, 2 * HD : GD],
        )
        wq_sb = P.tile([128, NDK * 128], BF16, tag="wq")
        nc.sync.dma_start(
            wq_sb[:].rearrange("p (a g) -> p a g", a=NDK),
            wq.rearrange("(a p) g -> p a g", p=128)[:, :, 0:128],
        )
        wv_sb = P.tile([128, NDK * GD], BF16, tag="wv")
        nc.gpsimd.dma_start(
            wv_sb[:].rearrange("p (a g) -> p a g", a=NDK),
            wv.rearrange("(a p) g -> p a g", p=128),
        )
        # wp: h0 rows at partitions 0:64 and h1 at 64:128 of block 0 (so the
        # h0+h1 pair contracts as one K=128 matmul); h2 in block 1.
        wp_sb = P.tile([128, 2 * D], BF16, tag="wp")
        nc.sync.dma_start(wp_sb[0:HD, 0:D], wp[0:HD, :])
        nc.sync.dma_start(wp_sb[HD:128, 0:D], wp[HD : 2 * HD, :])
        nc.sync.dma_start(wp_sb[0:HD, D : 2 * D], wp[2 * HD : 3 * HD, :])

        qT = P.tile([128, 2 * Q], BF16, tag="qT")       # q^T: rows=[h0|h1], [h2]
        kT = P.tile([128, 2 * KP], BF16, tag="kT")      # k^T subsampled
        qT_dup = P.tile([128, 2 * Q], BF16, tag="qTd")  # row-halves swapped
        kT_dup = P.tile([128, 2 * KP], BF16, tag="kTd")
        vv = P.tile([128, HPG * NKT * 65], BF16, tag="v")  # v + ones col, [k, 65]/tile
        # attn out: block 0 rows 0:64 = h0, rows 64:128 = h1; block 1 = h2
        attn = P.tile([128, 2 * Q], BF16, tag="attn")

        # ones columns of the v tiles
        nc.vector.memset(vv[:, 64 : HPG * NKT * 65 : 65], 1.0)

        # ---- HAM warmup: junk matmuls with no DMA dependency -------------
        junk_w = P.tile([128, 128], BF16, tag="jw")
        nc.vector.memset(junk_w[:], 0.0)
        junk_m = P.tile([128, ESW], BF16, tag="jm")
        nc.vector.memset(junk_m[:], 0.0)
        jps = SCPS.tile([128, 1024], F32, tag="sc", name="jps")
        for _ in range(N_WARM):
            nc.tensor.matmul(
                jps[0:128, 0:ESW], junk_w[:], junk_m[:], start=True, stop=True
            )

        # ---- A: projections --------------------------------------------
        def a1_chunk(i):
            """kT m0-block (heads h0|h1) for one key chunk."""
            c0, csz = KCH[i]
            ps = SCPS.tile([128, 1024], F32, tag="sc", name=f"km0_{i}")
            for kt in range(NDK):
                base = kt * Q
                nc.tensor.matmul(
                    ps[0:128, 0:csz],
                    wk_sb[:, kt * 128 : (kt + 1) * 128],
                    xt[:, base + 2 * c0 : base + 2 * (c0 + csz) : 2],
                    start=(kt == 0),
                    stop=(kt == NDK - 1),
                )
            nc.vector.tensor_copy(kT[0:128, c0 : c0 + csz], ps[0:128, 0:csz])
            nc.gpsimd.dma_start(
                kT_dup[HD:128, c0 : c0 + csz], kT[0:HD, c0 : c0 + csz]
            )
            nc.gpsimd.dma_start(
                kT_dup[0:HD, c0 : c0 + csz], kT[HD:128, c0 : c0 + csz]
            )

        def a2_chunk(c):
            """Combined (q_h2 | k_h2) pass over one q chunk; q keeps all
            positions, k keeps the even ones."""
            qo, ql = CHS[c]
            ps = SCPS.tile([128, 1024], F32, tag="sc", name=f"qk2_{c}")
            for kt in range(NDK):
                nc.tensor.matmul(
                    ps[0:128, 0:ql],
                    wqk2[:, kt * 128 : (kt + 1) * 128],
                    xt[:, kt * Q + qo : kt * Q + qo + ql],
                    start=(kt == 0),
                    stop=(kt == NDK - 1),
                )
            nc.vector.tensor_copy(
                qT[0:HD, Q + qo : Q + qo + ql], ps[0:HD, 0:ql]
            )
            ko, kl = qo // 2, ql // 2
            nc.vector.tensor_copy(
                kT[0:HD, KP + ko : KP + ko + kl], ps[HD:128, 0:ql:2]
            )
            nc.gpsimd.dma_start(
                qT_dup[HD:128, Q + qo : Q + qo + ql],
                qT[0:HD, Q + qo : Q + qo + ql],
            )
            nc.gpsimd.dma_start(
                kT_dup[HD:128, KP + ko : KP + ko + kl],
                kT[0:HD, KP + ko : KP + ko + kl],
            )

        def qT_m0(c):
            """qT m0-block (heads h0|h1) for one q-chunk, plus dup swaps."""
            qo, ql = CHS[c]
            ps = SCPS.tile([128, 1024], F32, tag="sc", name=f"qm0_{c}")
            for kt in range(NDK):
                nc.tensor.matmul(
                    ps[0:128, 0:ql],
                    wq_sb[:, kt * 128 : (kt + 1) * 128],
                    xt[:, kt * Q + qo : kt * Q + qo + ql],
                    start=(kt == 0),
                    stop=(kt == NDK - 1),
                )
            nc.vector.tensor_copy(qT[0:128, qo : qo + ql], ps[0:128, 0:ql])
            nc.gpsimd.dma_start(
                qT_dup[HD:128, qo : qo + ql], qT[0:HD, qo : qo + ql]
            )
            nc.gpsimd.dma_start(
                qT_dup[0:HD, qo : qo + ql], qT[HD:128, qo : qo + ql]
            )

        def vv_group(kt_m):
            """v projection for one k-tile: [ksz, GD] = x_sub @ wv."""
            msz = _ksize(kt_m)
            ps = NUMPS.tile([128, 512], F32, tag="num", name=f"vv{kt_m}")
            for kt in range(NDK):
                base = kt * Q + 2 * (kt_m * 128)
                nc.tensor.matmul(
                    ps[0:msz, 0:GD],
                    xt[:, base : base + 2 * msz : 2],
                    wv_sb[:, kt * GD : (kt + 1) * GD],
                    start=(kt == 0),
                    stop=(kt == NDK - 1),
                )
            nc.vector.tensor_copy(
                vv[0:msz].rearrange("p (h s) -> p h s", h=HPG)[
                    :, :, kt_m * 65 : kt_m * 65 + 64
                ],
                ps[0:msz, 0:GD].rearrange("p (h s) -> p h s", h=HPG),
            )

        a1_chunk(0)
        qT_m0(0)
        qT_m0(1)

        # ---- B/C: attention over chunk pairs ----------------------------
        def es4(es_t):
            return es_t.rearrange("p (k j c) -> p k j c", k=NKT, j=2)

        def score_group(h, c, j, grp, es_t):
            """Scores for one (even,odd) k-tile group x one q-chunk, + exp."""
            qo, ql = CHS[c]
            blk, pb = _head_pos(h)
            scp = SCPS.tile([128, 1024], F32, tag="sc")
            pmax = _ksize(grp[-1])
            for i, kt in enumerate(grp):
                ksz = _ksize(kt)
                if kt % 2 == 0:
                    sk, sq, base = kT, qT, pb
                else:
                    sk, sq, base = kT_dup, qT_dup, HD - pb
                nc.tensor.matmul(
                    scp[0:ksz, i * 512 : i * 512 + ql],
                    sk[base : base + HD, blk * KP + kt * 128 : blk * KP + kt * 128 + ksz],
                    sq[base : base + HD, blk * Q + qo : blk * Q + qo + ql],
                    start=True,
                    stop=True,
                    tile_position=(base, 0),
                )
            e = es4(es_t)
            if len(grp) == 2:
                src_ = scp[0:pmax, 0:1024].rearrange("p (a b) -> p a b", b=512)[
                    :, :, 0:ql
                ]
                dst = e[0:pmax, grp[0] : grp[0] + 2, j, 0:ql]
            else:
                src_ = scp[0:pmax, 0:ql]
                dst = e[0:pmax, grp[0], j, 0:ql]
            nc.scalar.activation(dst, src_, AF.Exp, scale=SC)

        def pv_part(h, es_t, j, c):
            """P@V with ones-column denominator; reciprocal + broadcast."""
            ql = CHS[c][1]
            e = es4(es_t)
            num = NUMPS.tile([128, 512], F32, tag="num")
            for kt in range(NKT):
                ksz = _ksize(kt)
                slot = (h * NKT + kt) * 65
                nc.tensor.matmul(
                    num[0:65, 0:ql],
                    vv[0:ksz, slot : slot + 65],
                    e[0:ksz, kt, j, 0:ql],
                    start=(kt == 0),
                    stop=(kt == NKT - 1),
                )
            den = INV.tile([128, ESW], F32, tag="den")
            nc.vector.tensor_copy(den[0:1, 0:ql], num[64:65, 0:ql])
            inv = INV.tile([128, ESW], F32, tag="inv")
            nc.vector.reciprocal_approx_fast(inv[0:1, 0:ql], den[0:1, 0:ql])
            invb = INV.tile([128, ESW], F32, tag="invb", bufs=2)
            nc.gpsimd.partition_broadcast(
                invb[0:HD, 0:ql], inv[0:1, 0:ql], channels=HD
            )
            return num, invb

        def rep_mult(h, c, num, invb):
            qo, ql = CHS[c]
            if h == 0:
                dst = attn[0:64, qo : qo + ql]
            elif h == 1:
                dst = attn[64:128, qo : qo + ql]
            else:
                dst = attn[0:64, Q + qo : Q + qo + ql]
            nc.vector.tensor_tensor(
                dst, num[0:64, 0:ql], invb[0:HD, 0:ql], op=mybir.AluOpType.mult
            )

        def proj_one(c, m):
            qo, ql = CHS[c]
            pp = SCPS.tile([128, 1024], F32, tag="sc", name=f"pj{m}")
            nc.tensor.matmul(
                pp[0:128, 0:ql],
                wp_sb[0:128, m * 128 : (m + 1) * 128],
                attn[0:128, qo : qo + ql],
                start=True,
                stop=False,
            )
            nc.tensor.matmul(
                pp[0:128, 0:ql],
                wp_sb[0:HD, D + m * 128 : D + (m + 1) * 128],
                attn[0:HD, Q + qo : Q + qo + ql],
                start=False,
                stop=True,
            )
            ot = OT.tile([128, ESW], BF16, tag="ot")
            nc.vector.tensor_copy(ot[0:128, 0:ql], pp[0:128, 0:ql])
            nc.sync.dma_start(
                out[m * 128 : (m + 1) * 128, qo : qo + ql], ot[0:128, 0:ql]
            )

        def mk(f, *a):
            return lambda: f(*a)

        prev = None  # (chunks, es tiles) of the previous pair
        for p, chunks in enumerate(CPAIRS):
            es = [
                ES.tile([128, NKT * 2 * ESW], BF16, tag="es", name=f"es{p}_{h}")
                for h in range(HPG)
            ]
            fillers = []
            if p == 0:
                fillers += [mk(a1_chunk, 1), mk(a1_chunk, 2), mk(a1_chunk, 3)]
                fillers += [mk(a2_chunk, c) for c in range(NCH)]
                fillers += [mk(vv_group, k) for k in range(NKT)]
                fillers += [mk(qT_m0, c) for c in CPAIRS[1]]
            else:
                state = {}
                pchunks, pes_l = prev

                def mk_pv(hh, pes, jj, cc):
                    def f():
                        state[(hh, jj)] = pv_part(hh, pes, jj, cc)
                    return f

                def mk_rep(hh, cc, jj):
                    def f():
                        num, invb = state[(hh, jj)]
                        rep_mult(hh, cc, num, invb)
                    return f

                nxt = CPAIRS[p + 1] if p + 1 < len(CPAIRS) else ()
                qts = [mk(qT_m0, c) for c in nxt]
                for h in range(HPG):
                    if h < len(qts):
                        fillers.append(qts[h])
                    for j, c in enumerate(pchunks):
                        fillers.append(mk_pv(h, pes_l[h], j, c))
                    for j, c in enumerate(pchunks):
                        fillers.append(mk_rep(h, c, j))
                for j, c in enumerate(pchunks):
                    for m in range(NDK):
                        fillers.append(mk(proj_one, c, m))
            fi = 0
            for j, c in enumerate(chunks):
                for h in range(HPG):
                    for grp in EXP_GROUPS:
                        score_group(h, c, j, grp, es[h])
                        if fi < len(fillers):
                            fillers[fi]()
                            fi += 1
            while fi < len(fillers):
                fillers[fi]()
                fi += 1
            prev = (chunks, es)

        # tail: PV + normalize + project the final solo chunk
        pchunks, pes_l = prev
        c6 = pchunks[0]
        p0 = pv_part(0, pes_l[0], 0, c6)
        p1 = pv_part(1, pes_l[1], 0, c6)
        rep_mult(0, c6, *p0)
        p2 = pv_part(2, pes_l[2], 0, c6)
        rep_mult(1, c6, *p1)
        rep_mult(2, c6, *p2)
        for m in range(NDK):
            proj_one(c6, m)


def _get_nc():
    if "nc" not in _CACHE:
        _CACHE["nc"] = _build_nc()
    return _CACHE["nc"]


def kernel(x, W_qkv, W_proj, b_proj):
    nc = _get_nc()
    xTs = [
        np.ascontiguousarray(
            x[n].reshape(Q, D).astype(BFNP).T
        )
        for n in range(N)
    ]
    wqs, wks, wvs, wps = [], [], [], []
    for g in range(4):
        c0 = g * GD
        wqs.append(np.ascontiguousarray(W_qkv[:, c0 : c0 + GD].astype(BFNP)))
        wks.append(np.ascontiguousarray(W_qkv[:, D + c0 : D + c0 + GD].astype(BFNP)))
        wvs.append(
            np.ascontiguousarray(W_qkv[:, 2 * D + c0 : 2 * D + c0 + GD].astype(BFNP))
        )
        wps.append(np.ascontiguousarray(W_proj[c0 : c0 + GD, :].astype(BFNP)))
    in_maps = [
        {"xT": xTs[c // 4], "wq": wqs[c % 4], "wk": wks[c % 4],
         "wv": wvs[c % 4], "wp": wps[c % 4]}
        for c in range(N_CORES)
    ]
    res = run_bass_kernel_spmd(nc, in_maps, list(range(N_CORES)), trace=TRACE)
    if TRACE:
        LAST_RESULTS["exec_time_ns"] = res.exec_time_ns
        LAST_RESULTS["mean_exec_time_ns"] = res.mean_exec_time_ns
    out = np.empty((N, T, S, D), np.float32)
    for n in range(N):
        acc = res.results[4 * n]["out"].astype(np.float32)
        for g in range(1, 4):
            acc = acc + res.results[4 * n + g]["out"].astype(np.float32)
        out[n] = (acc.T + b_proj).reshape(T, S, D)
    return out


# revision 15
# speedup vs baseline: 1.1675x; 1.0656x over previous
"""Trainium2 Bass kernel for nn_AttentionSubsample (8-core SPMD).

Sharding: batch N=2 x 4 head-groups (3 heads each) -> 8 cores, no
collectives.  Each core computes q/k/v projections for its head group
(K/V on the stride-2 subsampled positions only), per-head attention with
softmax folded as exp -> denominator via an appended ones-column in V ->
divide, and its partial output projection in transposed layout.  The
host sums the 4 per-batch partials and adds the bias.

Schedule notes (the kernel is jointly ACT(exp)- and PE-bound):
 - q chunks are 512 wide (one PSUM bank per matmul): at F=512 the
   213 ns score matmul hides the 107 ns LDWEIGHTS of the opposite
   row-half, so the (even,odd) k-tile pairs overlap fully on the PE.
   The ragged 64-wide tail chunk keeps the pipeline drain short.
 - x^T DMAs first, issued from two engine queues (the sync sequencer
   alone takes ~0.7us per dma_start); weights follow.
 - All projections except the first kT chunk run as fillers between
   score groups; a junk-matmul warmup keeps the PE clock at 8/8 during
   the initial x DMA.
 - P@V for the previous chunk pair runs as 26-matmul contiguous
   bursts (amortizes the PE stream-switch penalty).
 - Softmax normalization: DVE reciprocal of the denominator row (via
   an SBUF copy - custom-DVE bit tricks must not read PSUM), GpSimd
   partition-broadcast, one DVE multiply -> normalized bf16 attention.

Layout notes:
 - The spatial stride-2 subsample of K/V equals taking even rows of the
   flattened [3136, 768] batch (196 is even), i.e. even columns of x^T.
 - All device matmuls run in bf16 (fp32 PSUM accumulation).
 - Output is written bf16 (partial sums; host accumulates in fp32).
"""

import sys

for _p in ("/opt/trn_rl_repo",):
    if _p not in sys.path:
        sys.path.insert(0, _p)

import numpy as np
import ml_dtypes

import concourse.bass as bass  # noqa: F401  (registers engines)
import concourse.tile as tile
from concourse import bacc, mybir
from concourse.bass_utils import run_bass_kernel_spmd

BFNP = ml_dtypes.bfloat16
F32 = mybir.dt.float32
BF16 = mybir.dt.bfloat16
AF = mybir.ActivationFunctionType

N, T, S, D = 2, 16, 196, 768
H, HD = 12, 64
Q = T * S              # 3136 query positions per batch
KP = T * (S // 2)      # 1568 subsampled key positions
HPG = 3                # heads per group (12 heads / 4 groups)
GD = HPG * HD          # 192 channels per head group
SC = (D // H) ** -0.5  # 0.125 attention scale
# Schraudolph exp2 constants: bf16 bits of exp(SC*s) ~= int16(A*s + B).
# The global 2^-c scale factor cancels in the softmax ratio, so only the
# mantissa-linear shape error (~1.8% rms) remains on offloaded k-tiles.
EXP_A = SC * float(np.log2(np.e)) * 128.0
EXP_B = (127.0 - 0.043) * 128.0
DVE_GRPS = frozenset({(2, 3), (8, 9)})
NKT = 13               # k tiles: 12 * 128 + 32
KTL = 32               # last k-tile height
NDK = D // 128         # 6 contraction tiles for the projections
N_CORES = 8
N_WARM = 52            # junk matmuls covering the x DMA (HAM warmup)
ESW = 448              # es slot width (q per chunk)

# q chunks
CHS = [(c * 448, 448) for c in range(7)]
NCH = len(CHS)
CPAIRS = [(0, 1), (2, 3), (4, 5), (6,)]
# exp groups over k-tiles: PSUM scores tile holds 2 banks (512-aligned)
EXP_GROUPS = [(0, 1), (2, 3), (4, 5), (6, 7), (8, 9), (10, 11), (12,)]
# key chunks for the kT m0 projection
KCH = [(0, 448), (448, 448), (896, 448), (1344, 224)]

TRACE = False          # test.py flips this for profiled runs
LAST_RESULTS = {}      # exec_time_ns etc. stashed here on traced runs

_CACHE = {}


def _ksize(kt):
    return 128 if kt < NKT - 1 else KTL


def _head_pos(h):
    """(block, partition base) of head h inside the 2-block qT/kT tiles."""
    return (0, 0) if h == 0 else ((0, 64) if h == 1 else (1, 0))


def _build_nc():
    nc = bacc.Bacc(
        "TRN2", target_bir_lowering=False, debug=False, num_devices=N_CORES
    )
    xT = nc.dram_tensor("xT", [D, Q], BF16, kind="ExternalInput").ap()
    wq = nc.dram_tensor("wq", [D, GD], BF16, kind="ExternalInput").ap()
    wk = nc.dram_tensor("wk", [D, GD], BF16, kind="ExternalInput").ap()
    wv = nc.dram_tensor("wv", [D, GD], BF16, kind="ExternalInput").ap()
    wp = nc.dram_tensor("wp", [GD, D], BF16, kind="ExternalInput").ap()
    out = nc.dram_tensor("out", [D, Q], BF16, kind="ExternalOutput").ap()

    with tile.TileContext(nc) as tc:
        _body(tc, xT, wq, wk, wv, wp, out)
    nc.compile()
    return nc


def _body(tc, xT, wq, wk, wv, wp, out):
    nc = tc.nc
    with (
        tc.tile_pool(name="persist", bufs=1) as P,
        tc.tile_pool(name="es", bufs=4) as ES,
        tc.tile_pool(name="inv", bufs=2) as INV,
        tc.tile_pool(name="ot", bufs=3) as OT,
        tc.tile_pool(name="scps", bufs=3, space="PSUM") as SCPS,
        tc.tile_pool(name="numps", bufs=2, space="PSUM") as NUMPS,
    ):
        # ---- persistent SBUF tensors -------------------------------------
        # x^T gates everything: DMA it first, issue from two engine queues.
        xt = P.tile([128, NDK * Q], BF16, tag="xt")
        for kt in range(NDK):
            for hh in range(2):
                nc.sync.dma_start(
                    xt[:, kt * Q + hh * (Q // 2) : kt * Q + (hh + 1) * (Q // 2)],
                    xT[kt * 128 : (kt + 1) * 128, hh * (Q // 2) : (hh + 1) * (Q // 2)],
                )
        wk_sb = P.tile([128, NDK * 128], BF16, tag="wk")
        nc.sync.dma_start(
            wk_sb[:].rearrange("p (a g) -> p a g", a=NDK),
            wk.rearrange("(a p) g -> p a g", p=128)[:, :, 0:128],
        )
        # combined m1 weights: cols [wq_h2 | wk_h2] per D-ktile
        wqk2 = P.tile([128, NDK * 128], BF16, tag="wqk2")
        nc.gpsimd.dma_start(
            wqk2[:].rearrange("p (a g) -> p a g", a=NDK)[:, :, 0:HD],
            wq.rearrange("(a p) g -> p a g", p=128)[:, :, 2 * HD : GD],
        )
        nc.gpsimd.dma_start(
            wqk2[:].rearrange("p (a g) -> p a g", a=NDK)[:, :, HD:128],
            wk.rearrange("(a p) g -> p a g", p=128)[:, :, 2 * HD : GD],
        )
        wq_sb = P.tile([128, NDK * 128], BF16, tag="wq")
        nc.sync.dma_start(
            wq_sb[:].rearrange("p (a g) -> p a g", a=NDK),
            wq.rearrange("(a p) g -> p a g", p=128)[:, :, 0:128],
        )
        wv_sb = P.tile([128, NDK * GD], BF16, tag="wv")
        nc.gpsimd.dma_start(
            wv_sb[:].rearrange("p (a g) -> p a g", a=NDK),
            wv.rearrange("(a p) g -> p a g", p=128),
        )
        # wp: h0 rows at partitions 0:64 and h1 at 64:128 of block 0 (so the
        # h0+h1 pair contracts as one K=128 matmul); h2 in block 1.
        wp_sb = P.tile([128, 2 * D], BF16, tag="wp")
        nc.sync.dma_start(wp_sb[0:HD, 0:D], wp[0:HD, :])
        nc.sync.dma_start(wp_sb[HD:128, 0:D], wp[HD : 2 * HD, :])
        nc.sync.dma_start(wp_sb[0:HD, D : 2 * D], wp[2 * HD : 3 * HD, :])

        qT = P.tile([128, 2 * Q], BF16, tag="qT")       # q^T: rows=[h0|h1], [h2]
        kT = P.tile([128, 2 * KP], BF16, tag="kT")      # k^T subsampled
        qT_dup = P.tile([128, 2 * Q], BF16, tag="qTd")  # row-halves swapped
        kT_dup = P.tile([128, 2 * KP], BF16, tag="kTd")
        vv = P.tile([128, HPG * NKT * 65], BF16, tag="v")  # v + ones col, [k, 65]/tile
        # attn out: block 0 rows 0:64 = h0, rows 64:128 = h1; block 1 = h2
        attn = P.tile([128, 2 * Q], BF16, tag="attn")

        # ones columns of the v tiles
        nc.vector.memset(vv[:, 64 : HPG * NKT * 65 : 65], 1.0)

        # ---- HAM warmup: junk matmuls with no DMA dependency -------------
        junk_w = P.tile([128, 128], BF16, tag="jw")
        nc.vector.memset(junk_w[:], 0.0)
        junk_m = P.tile([128, ESW], BF16, tag="jm")
        nc.vector.memset(junk_m[:], 0.0)
        jps = SCPS.tile([128, 1024], F32, tag="sc", name="jps")
        for _ in range(N_WARM):
            nc.tensor.matmul(
                jps[0:128, 0:ESW], junk_w[:], junk_m[:], start=True, stop=True
            )

        # ---- A: projections --------------------------------------------
        def a1_chunk(i):
            """kT m0-block (heads h0|h1) for one key chunk."""
            c0, csz = KCH[i]
            ps = SCPS.tile([128, 1024], F32, tag="sc", name=f"km0_{i}")
            for kt in range(NDK):
                base = kt * Q
                nc.tensor.matmul(
                    ps[0:128, 0:csz],
                    wk_sb[:, kt * 128 : (kt + 1) * 128],
                    xt[:, base + 2 * c0 : base + 2 * (c0 + csz) : 2],
                    start=(kt == 0),
                    stop=(kt == NDK - 1),
                )
            nc.vector.tensor_copy(kT[0:128, c0 : c0 + csz], ps[0:128, 0:csz])
            nc.gpsimd.dma_start(
                kT_dup[HD:128, c0 : c0 + csz], kT[0:HD, c0 : c0 + csz]
            )
            nc.gpsimd.dma_start(
                kT_dup[0:HD, c0 : c0 + csz], kT[HD:128, c0 : c0 + csz]
            )

        def a2_chunk(c):
            """Combined (q_h2 | k_h2) pass over one q chunk; q keeps all
            positions, k keeps the even ones."""
            qo, ql = CHS[c]
            ps = SCPS.tile([128, 1024], F32, tag="sc", name=f"qk2_{c}")
            for kt in range(NDK):
                nc.tensor.matmul(
                    ps[0:128, 0:ql],
                    wqk2[:, kt * 128 : (kt + 1) * 128],
                    xt[:, kt * Q + qo : kt * Q + qo + ql],
                    start=(kt == 0),
                    stop=(kt == NDK - 1),
                )
            nc.vector.tensor_copy(
                qT[0:HD, Q + qo : Q + qo + ql], ps[0:HD, 0:ql]
            )
            ko, kl = qo // 2, ql // 2
            nc.vector.tensor_copy(
                kT[0:HD, KP + ko : KP + ko + kl], ps[HD:128, 0:ql:2]
            )
            nc.gpsimd.dma_start(
                qT_dup[HD:128, Q + qo : Q + qo + ql],
                qT[0:HD, Q + qo : Q + qo + ql],
            )
            nc.gpsimd.dma_start(
                kT_dup[HD:128, KP + ko : KP + ko + kl],
                kT[0:HD, KP + ko : KP + ko + kl],
            )

        def qT_m0(c):
            """qT m0-block (heads h0|h1) for one q-chunk, plus dup swaps."""
            qo, ql = CHS[c]
            ps = SCPS.tile([128, 1024], F32, tag="sc", name=f"qm0_{c}")
            for kt in range(NDK):
                nc.tensor.matmul(
                    ps[0:128, 0:ql],
                    wq_sb[:, kt * 128 : (kt + 1) * 128],
                    xt[:, kt * Q + qo : kt * Q + qo + ql],
                    start=(kt == 0),
                    stop=(kt == NDK - 1),
                )
            nc.vector.tensor_copy(qT[0:128, qo : qo + ql], ps[0:128, 0:ql])
            nc.gpsimd.dma_start(
                qT_dup[HD:128, qo : qo + ql], qT[0:HD, qo : qo + ql]
            )
            nc.gpsimd.dma_start(
                qT_dup[0:HD, qo : qo + ql], qT[HD:128, qo : qo + ql]
            )

        def vv_group(kt_m):
            """v projection for one k-tile: [ksz, GD] = x_sub @ wv."""
            msz = _ksize(kt_m)
            ps = NUMPS.tile([128, 512], F32, tag="num", name=f"vv{kt_m}")
            for kt in range(NDK):
                base = kt * Q + 2 * (kt_m * 128)
                nc.tensor.matmul(
                    ps[0:msz, 0:GD],
                    xt[:, base : base + 2 * msz : 2],
                    wv_sb[:, kt * GD : (kt + 1) * GD],
                    start=(kt == 0),
                    stop=(kt == NDK - 1),
                )
            nc.vector.tensor_copy(
                vv[0:msz].rearrange("p (h s) -> p h s", h=HPG)[
                    :, :, kt_m * 65 : kt_m * 65 + 64
                ],
                ps[0:msz, 0:GD].rearrange("p (h s) -> p h s", h=HPG),
            )

        a1_chunk(0)
        qT_m0(0)
        qT_m0(1)

        # ---- B/C: attention over chunk pairs ----------------------------
        def es4(es_t):
            return es_t.rearrange("p (k j c) -> p k j c", k=NKT, j=2)

        def score_group(h, c, j, grp, es_t):
            """Scores for one (even,odd) k-tile group x one q-chunk, + exp."""
            qo, ql = CHS[c]
            blk, pb = _head_pos(h)
            scp = SCPS.tile([128, 1024], F32, tag="sc")
            pmax = _ksize(grp[-1])
            for i, kt in enumerate(grp):
                ksz = _ksize(kt)
                if kt % 2 == 0:
                    sk, sq, base = kT, qT, pb
                else:
                    sk, sq, base = kT_dup, qT_dup, HD - pb
                nc.tensor.matmul(
                    scp[0:ksz, i * 512 : i * 512 + ql],
                    sk[base : base + HD, blk * KP + kt * 128 : blk * KP + kt * 128 + ksz],
                    sq[base : base + HD, blk * Q + qo : blk * Q + qo + ql],
                    start=True,
                    stop=True,
                    tile_position=(base, 0),
                )
            e = es4(es_t)
            if len(grp) == 2:
                src_ = scp[0:pmax, 0:1024].rearrange("p (a b) -> p a b", b=512)[
                    :, :, 0:ql
                ]
                dst = e[0:pmax, grp[0] : grp[0] + 2, j, 0:ql]
            else:
                src_ = scp[0:pmax, 0:ql]
                dst = e[0:pmax, grp[0], j, 0:ql]
            if grp in DVE_GRPS:
                # fast-exp on DVE: write bf16 BITS via the int16 view
                nc.vector.tensor_scalar(
                    dst.bitcast(mybir.dt.int16), src_, EXP_A, EXP_B,
                    op0=mybir.AluOpType.mult, op1=mybir.AluOpType.add,
                )
            else:
                nc.scalar.activation(dst, src_, AF.Exp, scale=SC)

        def pv_part(h, es_t, j, c):
            """P@V with ones-column denominator; reciprocal + broadcast."""
            ql = CHS[c][1]
            e = es4(es_t)
            num = NUMPS.tile([128, 512], F32, tag="num")
            for kt in range(NKT):
                ksz = _ksize(kt)
                slot = (h * NKT + kt) * 65
                nc.tensor.matmul(
                    num[0:65, 0:ql],
                    vv[0:ksz, slot : slot + 65],
                    e[0:ksz, kt, j, 0:ql],
                    start=(kt == 0),
                    stop=(kt == NKT - 1),
                )
            den = INV.tile([128, ESW], F32, tag="den")
            nc.vector.tensor_copy(den[0:1, 0:ql], num[64:65, 0:ql])
            inv = INV.tile([128, ESW], F32, tag="inv")
            nc.vector.reciprocal_approx_fast(inv[0:1, 0:ql], den[0:1, 0:ql])
            invb = INV.tile([128, ESW], F32, tag="invb", bufs=2)
            nc.gpsimd.partition_broadcast(
                invb[0:HD, 0:ql], inv[0:1, 0:ql], channels=HD
            )
            return num, invb

        def rep_mult(h, c, num, invb):
            qo, ql = CHS[c]
            if h == 0:
                dst = attn[0:64, qo : qo + ql]
            elif h == 1:
                dst = attn[64:128, qo : qo + ql]
            else:
                dst = attn[0:64, Q + qo : Q + qo + ql]
            nc.vector.tensor_tensor(
                dst, num[0:64, 0:ql], invb[0:HD, 0:ql], op=mybir.AluOpType.mult
            )

        def proj_one(c, m):
            qo, ql = CHS[c]
            pp = SCPS.tile([128, 1024], F32, tag="sc", name=f"pj{m}")
            nc.tensor.matmul(
                pp[0:128, 0:ql],
                wp_sb[0:128, m * 128 : (m + 1) * 128],
                attn[0:128, qo : qo + ql],
                start=True,
                stop=False,
            )
            nc.tensor.matmul(
                pp[0:128, 0:ql],
                wp_sb[0:HD, D + m * 128 : D + (m + 1) * 128],
                attn[0:HD, Q + qo : Q + qo + ql],
                start=False,
                stop=True,
            )
            ot = OT.tile([128, ESW], BF16, tag="ot")
            nc.vector.tensor_copy(ot[0:128, 0:ql], pp[0:128, 0:ql])
            nc.sync.dma_start(
                out[m * 128 : (m + 1) * 128, qo : qo + ql], ot[0:128, 0:ql]
            )

        def mk(f, *a):
            return lambda: f(*a)

        prev = None  # (chunks, es tiles) of the previous pair
        for p, chunks in enumerate(CPAIRS):
            es = [
                ES.tile([128, NKT * 2 * ESW], BF16, tag="es", name=f"es{p}_{h}")
                for h in range(HPG)
            ]
            fillers = []
            if p == 0:
                fillers += [mk(a1_chunk, 1), mk(a1_chunk, 2), mk(a1_chunk, 3)]
                fillers += [mk(a2_chunk, c) for c in range(NCH)]
                fillers += [mk(vv_group, k) for k in range(NKT)]
                fillers += [mk(qT_m0, c) for c in CPAIRS[1]]
            else:
                state = {}
                pchunks, pes_l = prev

                def mk_pv(hh, pes, jj, cc):
                    def f():
                        state[(hh, jj)] = pv_part(hh, pes, jj, cc)
                    return f

                def mk_rep(hh, cc, jj):
                    def f():
                        num, invb = state[(hh, jj)]
                        rep_mult(hh, cc, num, invb)
                    return f

                nxt = CPAIRS[p + 1] if p + 1 < len(CPAIRS) else ()
                qts = [mk(qT_m0, c) for c in nxt]
                for h in range(HPG):
                    if h < len(qts):
                        fillers.append(qts[h])
                    for j, c in enumerate(pchunks):
                        fillers.append(mk_pv(h, pes_l[h], j, c))
                    for j, c in enumerate(pchunks):
                        fillers.append(mk_rep(h, c, j))
                for j, c in enumerate(pchunks):
                    for m in range(NDK):
                        fillers.append(mk(proj_one, c, m))
            fi = 0
            for j, c in enumerate(chunks):
                for h in range(HPG):
                    for grp in EXP_GROUPS:
                        score_group(h, c, j, grp, es[h])
                        if fi < len(fillers):
                            fillers[fi]()
                            fi += 1
            while fi < len(fillers):
                fillers[fi]()
                fi += 1
            prev = (chunks, es)

        # tail: PV + normalize + project the final solo chunk
        pchunks, pes_l = prev
        c6 = pchunks[0]
        p0 = pv_part(0, pes_l[0], 0, c6)
        p1 = pv_part(1, pes_l[1], 0, c6)
        rep_mult(0, c6, *p0)
        p2 = pv_part(2, pes_l[2], 0, c6)
        rep_mult(1, c6, *p1)
        rep_mult(2, c6, *p2)
        for m in range(NDK):
            proj_one(c6, m)


def _get_nc():
    if "nc" not in _CACHE:
        _CACHE["nc"] = _build_nc()
    return _CACHE["nc"]


def kernel(x, W_qkv, W_proj, b_proj):
    nc = _get_nc()
    xTs = [
        np.ascontiguousarray(
            x[n].reshape(Q, D).astype(BFNP).T
        )
        for n in range(N)
    ]
    wqs, wks, wvs, wps = [], [], [], []
    for g in range(4):
        c0 = g * GD
        wqs.append(np.ascontiguousarray(W_qkv[:, c0 : c0 + GD].astype(BFNP)))
        wks.append(np.ascontiguousarray(W_qkv[:, D + c0 : D + c0 + GD].astype(BFNP)))
        wvs.append(
            np.ascontiguousarray(W_qkv[:, 2 * D + c0 : 2 * D + c0 + GD].astype(BFNP))
        )
        wps.append(np.ascontiguousarray(W_proj[c0 : c0 + GD, :].astype(BFNP)))
    in_maps = [
        {"xT": xTs[c // 4], "wq": wqs[c % 4], "wk": wks[c % 4],
         "wv": wvs[c % 4], "wp": wps[c % 4]}
        for c in range(N_CORES)
    ]
    res = run_bass_kernel_spmd(nc, in_maps, list(range(N_CORES)), trace=TRACE)
    if TRACE:
        LAST_RESULTS["exec_time_ns"] = res.exec_time_ns
        LAST_RESULTS["mean_exec_time_ns"] = res.mean_exec_time_ns
    out = np.empty((N, T, S, D), np.float32)
    for n in range(N):
        acc = res.results[4 * n]["out"].astype(np.float32)
        for g in range(1, 4):
            acc = acc + res.results[4 * n + g]["out"].astype(np.float32)
        out[n] = (acc.T + b_proj).reshape(T, S, D)
    return out
